# revision 26
# baseline (speedup 1.0000x reference)
"""Trainium2 Bass kernel for BasicAttention (Bahdanau-style additive attention).

Math (per batch row b):
    target  = x @ W_in.T                                   [B, D]
    source  = context @ W_c.T + b_c                        [B, S, D]
    attn    = tanh(target[:, None, :] + source)            [B, S, D]
    scores  = attn @ w_v                                   [B, S]
    attn_w  = softmax(scores, axis=S)                      [B, S]
    weighted= attn_w @ context                             [B, D]
    h_tilde = tanh(concat([weighted, x], -1) @ W_out.T)    [B, D]

Sharding: data-parallel over batch. 8 cores x 4 batch rows, full weights
replicated per core, no collectives. Compute in bf16 (fp32 accumulation in
PSUM); verified numerically to ~0.2-0.4% of output scale vs the fp32
reference.

Per-core dataflow:
  - W_c / W_in / W_out are cast to bf16, bounced through DRAM scratch and
    re-read with the DMA xbar transpose so the contraction dim lands on SBUF
    partitions.
  - context is cast-DMA'd (SWDGE fp32->bf16) into SBUF natural tiles
    [s=128p, d] (kept for the weighted sum), written to DRAM scratch, and
    transpose-read as [d=128p, s] tiles for the source matmul.
  - source psum tiles are [s=128, e=512]; the (target+b_c) bias row enters
    via a K=1 matmul with a ones row; tanh on ACT writes bf16 attn tiles.
  - scores = sum_e attn*w_v via DVE tensor_tensor_reduce -> [s=128, 1]
    column tiles (exactly the layout the weighted matmul needs).
  - softmax without max-subtraction (|scores| <= sum|w_v| ~ 16, exp is safe
    in fp32); the partition-sum of exp uses a ones[128,128] matmul which
    also leaves the total replicated across partitions for normalizing.
  - weighted and the output GEMM run with per-batch columns as stationary
    operands; small row<->column transposes go through the PE transpose path.
"""

import numpy as np

B, S, D = 32, 2048, 1024
N_CORES = 8
BL = B // N_CORES  # batch rows per core
ST = S // 128      # s-tiles of 128 per batch
SC = S // 512      # s-chunks of 512 per batch
DT = D // 128      # d/e tiles of 128
KT = 2 * D // 128  # k tiles for the output GEMM

_CACHE = {}


def _build_nc():
    from contextlib import ExitStack

    import concourse.bacc as bacc
    import concourse.bass as bass
    import concourse.tile as tile
    from concourse import mybir
    from concourse.masks import make_identity

    fp32 = mybir.dt.float32
    bf16 = mybir.dt.bfloat16
    AF = mybir.ActivationFunctionType
    ALU = mybir.AluOpType

    nc = bacc.Bacc(
        "TRN2",
        target_bir_lowering=False,
        debug=False,
        enable_asserts=False,
        num_devices=N_CORES,
    )

    x_d = nc.dram_tensor("x", [BL, D], fp32, kind="ExternalInput")
    ctx_d = nc.dram_tensor("context", [BL, S, D], fp32, kind="ExternalInput")
    win_d = nc.dram_tensor("W_in", [D, D], fp32, kind="ExternalInput")
    wc_d = nc.dram_tensor("W_c", [D, D], fp32, kind="ExternalInput")
    bc_d = nc.dram_tensor("b_c", [D], fp32, kind="ExternalInput")
    wv_d = nc.dram_tensor("w_v", [D], fp32, kind="ExternalInput")
    wout_d = nc.dram_tensor("W_out", [D, 2 * D], fp32, kind="ExternalInput")
    h_d = nc.dram_tensor("h_tilde", [BL, D], fp32, kind="ExternalOutput")
    aw_d = nc.dram_tensor("attn_w", [BL, S], fp32, kind="ExternalOutput")

    with tile.TileContext(nc) as tc, ExitStack() as ctx:
        consts = ctx.enter_context(tc.tile_pool(name="consts", bufs=1))
        weights = ctx.enter_context(tc.tile_pool(name="weights", bufs=1))
        dram = ctx.enter_context(tc.tile_pool(name="dram", bufs=1, space="DRAM"))
        ctxnatp = ctx.enter_context(tc.tile_pool(name="ctxnatp", bufs=2))
        ctxTp = ctx.enter_context(tc.tile_pool(name="ctxTp", bufs=12))
        attnp = ctx.enter_context(tc.tile_pool(name="attnp", bufs=4))
        smallp = ctx.enter_context(tc.tile_pool(name="smallp", bufs=4))
        stagep = ctx.enter_context(tc.tile_pool(name="stagep", bufs=3))
        batchp = ctx.enter_context(tc.tile_pool(name="batchp", bufs=2))
        psum1 = ctx.enter_context(tc.tile_pool(name="psum1", bufs=4, space="PSUM"))
        psum_m = ctx.enter_context(tc.tile_pool(name="psum_m", bufs=2, space="PSUM"))
        psum_w = ctx.enter_context(tc.tile_pool(name="psum_w", bufs=2, space="PSUM"))

        # ---------------- constants ----------------
        ones_row = consts.tile([1, 128], bf16)
        nc.vector.memset(ones_row, 1.0)
        ones_sq = consts.tile([128, 128], bf16)
        nc.vector.memset(ones_sq, 1.0)
        ident = consts.tile([128, 128], fp32)
        make_identity(nc, ident)

        bc_rows = consts.tile([BL, D], fp32)
        bc_ap = bc_d[:]
        nc.gpsimd.dma_start(
            out=bc_rows,
            in_=bass.AP(
                tensor=bc_ap.tensor, offset=bc_ap.offset, ap=[[0, BL], *bc_ap.ap]
            ),
        )

        wv_rep = consts.tile([128, D], bf16)
        wv_ap = wv_d[:]
        wv_bcast = bass.AP(
            tensor=wv_ap.tensor, offset=wv_ap.offset, ap=[[0, 128], *wv_ap.ap]
        )
        nc.gpsimd.dma_start(out=wv_rep, in_=wv_bcast)

        x_rows = consts.tile([BL, D], fp32)
        nc.sync.dma_start(out=x_rows, in_=x_d[:])

        # ---- weight transposes: cast to bf16, bounce via DRAM, xbar-read ----
        wcT = weights.tile([128, DT, D], bf16)       # [d_in, d_tile, e] resident
        woutTp = ctx.enter_context(tc.tile_pool(name="woutTp", bufs=4))

        xT_bf = consts.tile([128, DT, BL], bf16)
        # bias rows repacked onto partition 0 (matmul rhs must start at
        # partition 0/32/64, so a [b:b+1] row of a [BL, D] tile won't do)
        bias_bf = consts.tile([1, BL * D], bf16)
        weighted_flat = consts.tile([1, BL * D], bf16)  # on partition 0
        w_rows4 = consts.tile([BL, D], bf16)
        ident_bf = consts.tile([BL, BL], bf16)
        make_identity(nc, ident_bf)
        h_rows = consts.tile([BL, D], fp32)
        catT_bf = consts.tile([128, DT, BL], bf16)

        # persistent bf16 DRAM copies (column-transposed reads happen later)
        wout_scr = dram.tile([D, 2 * D], bf16, tag="woutscr")

        with tc.tile_pool(name="setup", bufs=1) as setupp:
            def cast_to_scratch(src, scr, n_rows, col0, n_cols):
                """scr[:, col0:col0+n_cols] <- bf16(src[:, col0:col0+n_cols])"""
                nat = setupp.tile([128, n_rows // 128, n_cols], bf16, tag="wnat")
                nc.gpsimd.dma_start(
                    out=nat,
                    in_=src[0:n_rows, col0 : col0 + n_cols].rearrange(
                        "(t p) c -> p t c", p=128
                    ),
                )
                nc.sync.dma_start(
                    out=scr[0:n_rows, col0 : col0 + n_cols].rearrange(
                        "(t p) c -> p t c", p=128
                    ),
                    in_=nat,
                )

            wc_scr = dram.tile([D, D], bf16, tag="wcscr")
            cast_to_scratch(wc_d, wc_scr, D, 0, D)
            for t in range(DT):
                nc.scalar.dma_start_transpose(
                    wcT[:, t, :], wc_scr[0:D, t * 128 : (t + 1) * 128]
                )
            cast_to_scratch(wout_d, wout_scr, D, 0, D)
            cast_to_scratch(wout_d, wout_scr, D, D, D)
            win_scr = dram.tile([D, D], bf16, tag="winscr")
            cast_to_scratch(win_d, win_scr, D, 0, D)

            # x^T columns (also reused as the second half of catT)
            for dt in range(DT):
                ps = psum_m.tile([128, BL], fp32, tag="psm")
                nc.tensor.transpose(
                    ps, x_rows[:, dt * 128 : (dt + 1) * 128], ident[:BL, :BL]
                )
                nc.vector.tensor_copy(xT_bf[:, dt, :], ps)

            # target rows + b_c -> bias rows (bf16); W_in^T streamed per tile
            bias_f32 = setupp.tile([BL, D], fp32, tag="biasf")
            ps_t = [psum_w.tile([BL, 512], fp32, tag="psw", name=f"ps_t{i}") for i in range(2)]
            for dt in range(DT):
                winT_t = setupp.tile([128, D], bf16, tag="winTt", bufs=3)
                nc.scalar.dma_start_transpose(
                    winT_t, win_scr[0:D, dt * 128 : (dt + 1) * 128]
                )
                for eh in range(2):
                    nc.tensor.matmul(
                        ps_t[eh],
                        lhsT=xT_bf[:, dt, :],
                        rhs=winT_t[:, eh * 512 : (eh + 1) * 512],
                        start=(dt == 0),
                        stop=(dt == DT - 1),
                    )
            for eh in range(2):
                nc.vector.tensor_add(
                    bias_f32[:, eh * 512 : (eh + 1) * 512],
                    ps_t[eh],
                    bc_rows[:, eh * 512 : (eh + 1) * 512],
                )
            # repack rows (partitions 0..BL-1) onto partition 0 via a DRAM
            # bounce (SBUF DMA sources must start at a supported partition)
            bias_scr = dram.tile([1, BL * D], bf16, tag="bscr")
            nc.gpsimd.dma_start(out=bias_scr, in_=bias_f32)
            nc.sync.dma_start(out=bias_bf, in_=bias_scr[:])

        # ---------------- main batch loop ----------------
        for b in range(BL):
            ctxnat = ctxnatp.tile([128, ST, D], bf16, tag="ctxnat")
            ctxscr = dram.tile([S, D], bf16, tag="ctxscr", bufs=2)
            for st in range(ST):
                stg = stagep.tile([128, D], fp32, tag="stg")
                nc.sync.dma_start(
                    out=stg, in_=ctx_d[b, st * 128 : (st + 1) * 128, :]
                )
                nc.vector.tensor_copy(ctxnat[:, st, :], stg)
            for sc in range(SC):
                nc.sync.dma_start(
                    out=ctxscr[sc * 512 : (sc + 1) * 512, :].rearrange(
                        "(t p) d -> p t d", p=128
                    ),
                    in_=ctxnat[:, sc * 4 : (sc + 1) * 4, :],
                )

            scores_cols = batchp.tile([128, ST], fp32, tag="scols")
            for sc in range(SC):
                ctxTs = []
                for dt in range(DT):
                    t = ctxTp.tile([128, 512], bf16, tag="ctxT")
                    nc.scalar.dma_start_transpose(
                        t, ctxscr[sc * 512 : (sc + 1) * 512, dt * 128 : (dt + 1) * 128]
                    )
                    ctxTs.append(t)
                for sb in range(4):
                    st = sc * 4 + sb
                    prod = smallp.tile([128, D], bf16, tag="prod")
                    for eh in range(2):
                        ps = psum1.tile([128, 512], fp32, tag="ps1")
                        nc.tensor.matmul(
                            ps,
                            lhsT=ones_row,
                            rhs=bias_bf[0:1, b * D + eh * 512 : b * D + (eh + 1) * 512],
                            start=True,
                            stop=False,
                        )
                        for dt in range(DT):
                            nc.tensor.matmul(
                                ps,
                                lhsT=ctxTs[dt][:, sb * 128 : (sb + 1) * 128],
                                rhs=wcT[:, dt, eh * 512 : (eh + 1) * 512],
                                start=False,
                                stop=(dt == DT - 1),
                            )
                        attn_sb = attnp.tile([128, 512], bf16, tag="attn")
                        nc.scalar.activation(attn_sb, ps, AF.Tanh)
                        nc.vector.tensor_mul(
                            prod[:, eh * 512 : (eh + 1) * 512],
                            attn_sb,
                            wv_rep[:, eh * 512 : (eh + 1) * 512],
                        )
                    nc.vector.reduce_sum(
                        scores_cols[:, st : st + 1], prod, axis=mybir.AxisListType.X
                    )

            # softmax over S (no max subtraction: |scores| <= sum|w_v| ~ 16)
            p_cols = batchp.tile([128, ST], fp32, tag="pcols")
            l_col = batchp.tile([128, 1], fp32, tag="lcol")
            nc.scalar.activation(p_cols, scores_cols, AF.Exp, accum_out=l_col)
            l_bf = batchp.tile([128, 1], bf16, tag="lbf")
            nc.vector.tensor_copy(l_bf, l_col)
            ps_l = psum_m.tile([128, 1], fp32, tag="psm")
            nc.tensor.matmul(ps_l, lhsT=ones_sq, rhs=l_bf, start=True, stop=True)
            rl = batchp.tile([128, 1], fp32, tag="rl")
            nc.vector.reciprocal(rl, ps_l)
            aw_cols = batchp.tile([128, ST], fp32, tag="awcols")
            nc.vector.tensor_scalar_mul(aw_cols, p_cols, rl)
            aw_bf = batchp.tile([128, ST], bf16, tag="awbf")
            nc.vector.tensor_copy(aw_bf, aw_cols)

            ps_awT = psum_m.tile([ST, 128], fp32, tag="psm")
            nc.tensor.transpose(ps_awT, aw_cols, ident)
            aw_out = batchp.tile([ST, 128], fp32, tag="awrows")
            nc.vector.tensor_copy(aw_out, ps_awT)
            nc.sync.dma_start(
                out=aw_d[b].rearrange("(t p) -> t p", p=128), in_=aw_out
            )

            for dh in range(2):
                ps_w = psum_w.tile([1, 512], fp32, tag="psw")
                for st in range(ST):
                    nc.tensor.matmul(
                        ps_w,
                        lhsT=aw_bf[:, st : st + 1],
                        rhs=ctxnat[:, st, dh * 512 : (dh + 1) * 512],
                        start=(st == 0),
                        stop=(st == ST - 1),
                    )
                nc.vector.tensor_copy(
                    weighted_flat[0:1, b * D + dh * 512 : b * D + (dh + 1) * 512],
                    ps_w,
                )

        # ---------------- output GEMM ----------------
        # bounce weighted rows (all on partition 0) out to DRAM and back as
        # [BL, D] rows on partitions 0..BL-1 for the PE transposes
        wf_scr = dram.tile([1, BL * D], bf16, tag="wfscr")
        nc.sync.dma_start(out=wf_scr, in_=weighted_flat)
        nc.sync.dma_start(
            out=w_rows4,
            in_=wf_scr[:].rearrange("a (b d) -> b (a d)", b=BL),
        )
        for dt in range(DT):
            ps = psum_m.tile([128, BL], bf16, tag="psm")
            nc.tensor.transpose(
                ps, w_rows4[:, dt * 128 : (dt + 1) * 128], ident_bf
            )
            nc.vector.tensor_copy(catT_bf[:, dt, :], ps)
        ps_h = [psum_w.tile([BL, 512], fp32, tag="psw", name=f"ps_h{i}") for i in range(2)]
        for kt in range(KT):
            woutT_t = woutTp.tile([128, D], bf16, tag="woutTt")
            nc.scalar.dma_start_transpose(
                woutT_t, wout_scr[0:D, kt * 128 : (kt + 1) * 128]
            )
            lhsT = catT_bf[:, kt, :] if kt < DT else xT_bf[:, kt - DT, :]
            for oh in range(2):
                nc.tensor.matmul(
                    ps_h[oh],
                    lhsT=lhsT,
                    rhs=woutT_t[:, oh * 512 : (oh + 1) * 512],
                    start=(kt == 0),
                    stop=(kt == KT - 1),
                )
        for oh in range(2):
            nc.scalar.activation(h_rows[:, oh * 512 : (oh + 1) * 512], ps_h[oh], AF.Tanh)
        nc.sync.dma_start(out=h_d[:], in_=h_rows)

    nc.compile()
    return nc


def get_nc():
    if "nc" not in _CACHE:
        _CACHE["nc"] = _build_nc()
    return _CACHE["nc"]


def _make_in_maps(inputs):
    x = np.ascontiguousarray(np.asarray(inputs["x"], dtype=np.float32))
    context = np.ascontiguousarray(np.asarray(inputs["context"], dtype=np.float32))
    weights = {
        k: np.ascontiguousarray(np.asarray(inputs[k], dtype=np.float32))
        for k in ("W_in", "W_c", "b_c", "w_v", "W_out")
    }
    return [
        {
            "x": x[i * BL : (i + 1) * BL],
            "context": context[i * BL : (i + 1) * BL],
            **weights,
        }
        for i in range(N_CORES)
    ]


def kernel(x, context, W_in, W_c, b_c, w_v, W_out):
    from concourse.bass_utils import run_bass_kernel_spmd

    nc = get_nc()
    in_maps = _make_in_maps(
        dict(x=x, context=context, W_in=W_in, W_c=W_c, b_c=b_c, w_v=w_v, W_out=W_out)
    )
    res = run_bass_kernel_spmd(nc, in_maps, list(range(N_CORES)))
    h = np.concatenate([r["h_tilde"] for r in res.results], axis=0)
    aw = np.concatenate([r["attn_w"] for r in res.results], axis=0)
    return h, aw


if __name__ == "__main__":
    import reference as R

    inputs = {k: np.asarray(v) for k, v in R.setup_inputs().items()}
    h, aw = kernel(**inputs)
    print(h.shape, aw.shape, h.dtype, aw.dtype)


# revision 28
# speedup vs baseline: 1.0471x; 1.0471x over previous
"""Trainium2 Bass kernel for BasicAttention (Bahdanau-style additive attention).

Math (per batch row b):
    target  = x @ W_in.T                                   [B, D]
    source  = context @ W_c.T + b_c                        [B, S, D]
    attn    = tanh(target[:, None, :] + source)            [B, S, D]
    scores  = attn @ w_v                                   [B, S]
    attn_w  = softmax(scores, axis=S)                      [B, S]
    weighted= attn_w @ context                             [B, D]
    h_tilde = tanh(concat([weighted, x], -1) @ W_out.T)    [B, D]

Sharding: data-parallel over batch. 8 cores x 4 batch rows, full weights
replicated per core, no collectives. Compute in bf16 (fp32 accumulation in
PSUM); verified numerically to ~0.2-0.4% of output scale vs the fp32
reference.

Per-core dataflow:
  - W_c / W_in / W_out are cast to bf16, bounced through DRAM scratch and
    re-read with the DMA xbar transpose so the contraction dim lands on SBUF
    partitions.
  - context is cast-DMA'd (SWDGE fp32->bf16) into SBUF natural tiles
    [s=128p, d] (kept for the weighted sum), written to DRAM scratch, and
    transpose-read as [d=128p, s] tiles for the source matmul.
  - source psum tiles are [s=128, e=512]; the (target+b_c) bias row enters
    via a K=1 matmul with a ones row; tanh on ACT writes bf16 attn tiles.
  - scores = sum_e attn*w_v via DVE tensor_tensor_reduce -> [s=128, 1]
    column tiles (exactly the layout the weighted matmul needs).
  - softmax without max-subtraction (|scores| <= sum|w_v| ~ 16, exp is safe
    in fp32); the partition-sum of exp uses a ones[128,128] matmul which
    also leaves the total replicated across partitions for normalizing.
  - weighted and the output GEMM run with per-batch columns as stationary
    operands; small row<->column transposes go through the PE transpose path.
"""

import numpy as np

B, S, D = 32, 2048, 1024
N_CORES = 8
BL = B // N_CORES  # batch rows per core
ST = S // 128      # s-tiles of 128 per batch
SC = S // 512      # s-chunks of 512 per batch
DT = D // 128      # d/e tiles of 128
KT = 2 * D // 128  # k tiles for the output GEMM

_CACHE = {}


def _build_nc():
    from contextlib import ExitStack

    import concourse.bacc as bacc
    import concourse.bass as bass
    import concourse.tile as tile
    from concourse import mybir
    from concourse.masks import make_identity

    fp32 = mybir.dt.float32
    bf16 = mybir.dt.bfloat16
    AF = mybir.ActivationFunctionType
    ALU = mybir.AluOpType

    nc = bacc.Bacc(
        "TRN2",
        target_bir_lowering=False,
        debug=False,
        enable_asserts=False,
        num_devices=N_CORES,
    )

    x_d = nc.dram_tensor("x", [BL, D], fp32, kind="ExternalInput")
    ctx_d = nc.dram_tensor("context", [BL, S, D], fp32, kind="ExternalInput")
    win_d = nc.dram_tensor("W_in", [D, D], fp32, kind="ExternalInput")
    wc_d = nc.dram_tensor("W_c", [D, D], fp32, kind="ExternalInput")
    bc_d = nc.dram_tensor("b_c", [D], fp32, kind="ExternalInput")
    wv_d = nc.dram_tensor("w_v", [D], fp32, kind="ExternalInput")
    wout_d = nc.dram_tensor("W_out", [D, 2 * D], fp32, kind="ExternalInput")
    h_d = nc.dram_tensor("h_tilde", [BL, D], fp32, kind="ExternalOutput")
    aw_d = nc.dram_tensor("attn_w", [BL, S], fp32, kind="ExternalOutput")

    with tile.TileContext(nc) as tc, ExitStack() as ctx:
        consts = ctx.enter_context(tc.tile_pool(name="consts", bufs=1))
        weights = ctx.enter_context(tc.tile_pool(name="weights", bufs=1))
        dram = ctx.enter_context(tc.tile_pool(name="dram", bufs=1, space="DRAM"))
        ctxnatp = ctx.enter_context(tc.tile_pool(name="ctxnatp", bufs=2))
        ctxTp = ctx.enter_context(tc.tile_pool(name="ctxTp", bufs=12))
        attnp = ctx.enter_context(tc.tile_pool(name="attnp", bufs=4))
        smallp = ctx.enter_context(tc.tile_pool(name="smallp", bufs=4))
        stagep = ctx.enter_context(tc.tile_pool(name="stagep", bufs=3))
        batchp = ctx.enter_context(tc.tile_pool(name="batchp", bufs=2))
        psum1 = ctx.enter_context(tc.tile_pool(name="psum1", bufs=4, space="PSUM"))
        psum_m = ctx.enter_context(tc.tile_pool(name="psum_m", bufs=2, space="PSUM"))
        psum_w = ctx.enter_context(tc.tile_pool(name="psum_w", bufs=2, space="PSUM"))

        # ---------------- constants ----------------
        ones_row = consts.tile([1, 128], bf16)
        nc.vector.memset(ones_row, 1.0)
        ones_sq = consts.tile([128, 128], bf16)
        nc.vector.memset(ones_sq, 1.0)
        ident = consts.tile([128, 128], fp32)
        make_identity(nc, ident)

        bc_rows = consts.tile([BL, D], fp32)
        bc_ap = bc_d[:]
        nc.gpsimd.dma_start(
            out=bc_rows,
            in_=bass.AP(
                tensor=bc_ap.tensor, offset=bc_ap.offset, ap=[[0, BL], *bc_ap.ap]
            ),
        )

        wv_rep = consts.tile([128, D], bf16)
        wv_ap = wv_d[:]
        wv_bcast = bass.AP(
            tensor=wv_ap.tensor, offset=wv_ap.offset, ap=[[0, 128], *wv_ap.ap]
        )
        nc.gpsimd.dma_start(out=wv_rep, in_=wv_bcast)

        x_rows = consts.tile([BL, D], fp32)
        nc.sync.dma_start(out=x_rows, in_=x_d[:])

        # ---- weight transposes: cast to bf16, bounce via DRAM, xbar-read ----
        wcT = weights.tile([128, DT, D], bf16)       # [d_in, d_tile, e] resident
        woutTp = ctx.enter_context(tc.tile_pool(name="woutTp", bufs=4))

        xT_bf = consts.tile([128, DT, BL], bf16)
        # bias rows repacked onto partition 0 (matmul rhs must start at
        # partition 0/32/64, so a [b:b+1] row of a [BL, D] tile won't do)
        bias_bf = consts.tile([1, BL * D], bf16)
        weighted_flat = consts.tile([1, BL * D], bf16)  # on partition 0
        w_rows4 = consts.tile([BL, D], bf16)
        ident_bf = consts.tile([BL, BL], bf16)
        make_identity(nc, ident_bf)
        h_rows = consts.tile([BL, D], fp32)
        catT_bf = consts.tile([128, DT, BL], bf16)

        # persistent bf16 DRAM copies (column-transposed reads happen later)
        wout_scr = dram.tile([D, 2 * D], bf16, tag="woutscr")

        with tc.tile_pool(name="setup", bufs=1) as setupp:
            def cast_to_scratch(src, scr, n_rows, col0, n_cols):
                """scr[:, col0:col0+n_cols] <- bf16(src[:, col0:col0+n_cols])"""
                nat = setupp.tile([128, n_rows // 128, n_cols], bf16, tag="wnat")
                nc.gpsimd.dma_start(
                    out=nat,
                    in_=src[0:n_rows, col0 : col0 + n_cols].rearrange(
                        "(t p) c -> p t c", p=128
                    ),
                )
                nc.sync.dma_start(
                    out=scr[0:n_rows, col0 : col0 + n_cols].rearrange(
                        "(t p) c -> p t c", p=128
                    ),
                    in_=nat,
                )

            wc_scr = dram.tile([D, D], bf16, tag="wcscr")
            cast_to_scratch(wc_d, wc_scr, D, 0, D)
            for t in range(DT):
                nc.scalar.dma_start_transpose(
                    wcT[:, t, :], wc_scr[0:D, t * 128 : (t + 1) * 128]
                )
            cast_to_scratch(wout_d, wout_scr, D, 0, D)
            cast_to_scratch(wout_d, wout_scr, D, D, D)
            win_scr = dram.tile([D, D], bf16, tag="winscr")
            cast_to_scratch(win_d, win_scr, D, 0, D)

            # x^T columns (also reused as the second half of catT)
            for dt in range(DT):
                ps = psum_m.tile([128, BL], fp32, tag="psm")
                nc.tensor.transpose(
                    ps, x_rows[:, dt * 128 : (dt + 1) * 128], ident[:BL, :BL]
                )
                nc.vector.tensor_copy(xT_bf[:, dt, :], ps)

            # target rows + b_c -> bias rows (bf16); W_in^T streamed per tile
            bias_f32 = setupp.tile([BL, D], fp32, tag="biasf")
            ps_t = [psum_w.tile([BL, 512], fp32, tag="psw", name=f"ps_t{i}") for i in range(2)]
            for dt in range(DT):
                winT_t = setupp.tile([128, D], bf16, tag="winTt", bufs=3)
                nc.scalar.dma_start_transpose(
                    winT_t, win_scr[0:D, dt * 128 : (dt + 1) * 128]
                )
                for eh in range(2):
                    nc.tensor.matmul(
                        ps_t[eh],
                        lhsT=xT_bf[:, dt, :],
                        rhs=winT_t[:, eh * 512 : (eh + 1) * 512],
                        start=(dt == 0),
                        stop=(dt == DT - 1),
                    )
            for eh in range(2):
                nc.vector.tensor_add(
                    bias_f32[:, eh * 512 : (eh + 1) * 512],
                    ps_t[eh],
                    bc_rows[:, eh * 512 : (eh + 1) * 512],
                )
            # repack rows (partitions 0..BL-1) onto partition 0 via a DRAM
            # bounce (SBUF DMA sources must start at a supported partition)
            bias_scr = dram.tile([1, BL * D], bf16, tag="bscr")
            nc.gpsimd.dma_start(out=bias_scr, in_=bias_f32)
            nc.sync.dma_start(out=bias_bf, in_=bias_scr[:])

        # ---------------- main batch loop ----------------
        for b in range(BL):
            ctxnat = ctxnatp.tile([128, ST, D], bf16, tag="ctxnat")
            ctxscr = dram.tile([S, D], bf16, tag="ctxscr", bufs=2)
            for st in range(ST):
                stg = stagep.tile([128, D], fp32, tag="stg")
                nc.sync.dma_start(
                    out=stg, in_=ctx_d[b, st * 128 : (st + 1) * 128, :]
                )
                nc.vector.tensor_copy(ctxnat[:, st, :], stg)
            for sc in range(SC):
                nc.sync.dma_start(
                    out=ctxscr[sc * 512 : (sc + 1) * 512, :].rearrange(
                        "(t p) d -> p t d", p=128
                    ),
                    in_=ctxnat[:, sc * 4 : (sc + 1) * 4, :],
                )

            scores_cols = batchp.tile([128, ST], fp32, tag="scols")
            for sc in range(SC):
                ctxTs = []
                for dt in range(DT):
                    t = ctxTp.tile([128, 512], bf16, tag="ctxT")
                    nc.scalar.dma_start_transpose(
                        t, ctxscr[sc * 512 : (sc + 1) * 512, dt * 128 : (dt + 1) * 128]
                    )
                    ctxTs.append(t)
                for sb in range(4):
                    st = sc * 4 + sb
                    prod = smallp.tile([128, D], bf16, tag="prod")
                    # two psum tiles (e-halves) accumulated together so each
                    # stationary operand serves 2 matmuls (halves the
                    # drain-exposed weight swaps on the PE)
                    pss = [
                        psum1.tile([128, 512], fp32, tag="ps1", name=f"ps1_{eh}")
                        for eh in range(2)
                    ]
                    for eh in range(2):
                        nc.tensor.matmul(
                            pss[eh],
                            lhsT=ones_row,
                            rhs=bias_bf[0:1, b * D + eh * 512 : b * D + (eh + 1) * 512],
                            start=True,
                            stop=False,
                        )
                    for dt in range(DT):
                        lhsT = ctxTs[dt][:, sb * 128 : (sb + 1) * 128]
                        for eh in range(2):
                            nc.tensor.matmul(
                                pss[eh],
                                lhsT=lhsT,
                                rhs=wcT[:, dt, eh * 512 : (eh + 1) * 512],
                                start=False,
                                stop=(dt == DT - 1),
                            )
                    for eh in range(2):
                        attn_sb = attnp.tile([128, 512], bf16, tag="attn")
                        nc.scalar.activation(attn_sb, pss[eh], AF.Tanh)
                        nc.vector.tensor_mul(
                            prod[:, eh * 512 : (eh + 1) * 512],
                            attn_sb,
                            wv_rep[:, eh * 512 : (eh + 1) * 512],
                        )
                    nc.vector.reduce_sum(
                        scores_cols[:, st : st + 1], prod, axis=mybir.AxisListType.X
                    )

            # softmax over S (no max subtraction: |scores| <= sum|w_v| ~ 16)
            p_cols = batchp.tile([128, ST], fp32, tag="pcols")
            l_col = batchp.tile([128, 1], fp32, tag="lcol")
            nc.scalar.activation(p_cols, scores_cols, AF.Exp, accum_out=l_col)
            l_bf = batchp.tile([128, 1], bf16, tag="lbf")
            nc.vector.tensor_copy(l_bf, l_col)
            ps_l = psum_m.tile([128, 1], fp32, tag="psm")
            nc.tensor.matmul(ps_l, lhsT=ones_sq, rhs=l_bf, start=True, stop=True)
            rl = batchp.tile([128, 1], fp32, tag="rl")
            nc.vector.reciprocal(rl, ps_l)
            aw_cols = batchp.tile([128, ST], fp32, tag="awcols")
            nc.vector.tensor_scalar_mul(aw_cols, p_cols, rl)
            aw_bf = batchp.tile([128, ST], bf16, tag="awbf")
            nc.vector.tensor_copy(aw_bf, aw_cols)

            ps_awT = psum_m.tile([ST, 128], fp32, tag="psm")
            nc.tensor.transpose(ps_awT, aw_cols, ident)
            aw_out = batchp.tile([ST, 128], fp32, tag="awrows")
            nc.vector.tensor_copy(aw_out, ps_awT)
            nc.sync.dma_start(
                out=aw_d[b].rearrange("(t p) -> t p", p=128), in_=aw_out
            )

            ps_ws = [
                psum_w.tile([1, 512], fp32, tag="psw", name=f"ps_w{dh}")
                for dh in range(2)
            ]
            for st in range(ST):
                for dh in range(2):
                    nc.tensor.matmul(
                        ps_ws[dh],
                        lhsT=aw_bf[:, st : st + 1],
                        rhs=ctxnat[:, st, dh * 512 : (dh + 1) * 512],
                        start=(st == 0),
                        stop=(st == ST - 1),
                    )
            for dh in range(2):
                nc.vector.tensor_copy(
                    weighted_flat[0:1, b * D + dh * 512 : b * D + (dh + 1) * 512],
                    ps_ws[dh],
                )

        # ---------------- output GEMM ----------------
        # bounce weighted rows (all on partition 0) out to DRAM and back as
        # [BL, D] rows on partitions 0..BL-1 for the PE transposes
        wf_scr = dram.tile([1, BL * D], bf16, tag="wfscr")
        nc.sync.dma_start(out=wf_scr, in_=weighted_flat)
        nc.sync.dma_start(
            out=w_rows4,
            in_=wf_scr[:].rearrange("a (b d) -> b (a d)", b=BL),
        )
        for dt in range(DT):
            ps = psum_m.tile([128, BL], bf16, tag="psm")
            nc.tensor.transpose(
                ps, w_rows4[:, dt * 128 : (dt + 1) * 128], ident_bf
            )
            nc.vector.tensor_copy(catT_bf[:, dt, :], ps)
        ps_h = [psum_w.tile([BL, 512], fp32, tag="psw", name=f"ps_h{i}") for i in range(2)]
        for kt in range(KT):
            woutT_t = woutTp.tile([128, D], bf16, tag="woutTt")
            nc.scalar.dma_start_transpose(
                woutT_t, wout_scr[0:D, kt * 128 : (kt + 1) * 128]
            )
            lhsT = catT_bf[:, kt, :] if kt < DT else xT_bf[:, kt - DT, :]
            for oh in range(2):
                nc.tensor.matmul(
                    ps_h[oh],
                    lhsT=lhsT,
                    rhs=woutT_t[:, oh * 512 : (oh + 1) * 512],
                    start=(kt == 0),
                    stop=(kt == KT - 1),
                )
        for oh in range(2):
            nc.scalar.activation(h_rows[:, oh * 512 : (oh + 1) * 512], ps_h[oh], AF.Tanh)
        nc.sync.dma_start(out=h_d[:], in_=h_rows)

    nc.compile()
    return nc


def get_nc():
    if "nc" not in _CACHE:
        _CACHE["nc"] = _build_nc()
    return _CACHE["nc"]


def _make_in_maps(inputs):
    x = np.ascontiguousarray(np.asarray(inputs["x"], dtype=np.float32))
    context = np.ascontiguousarray(np.asarray(inputs["context"], dtype=np.float32))
    weights = {
        k: np.ascontiguousarray(np.asarray(inputs[k], dtype=np.float32))
        for k in ("W_in", "W_c", "b_c", "w_v", "W_out")
    }
    return [
        {
            "x": x[i * BL : (i + 1) * BL],
            "context": context[i * BL : (i + 1) * BL],
            **weights,
        }
        for i in range(N_CORES)
    ]


def kernel(x, context, W_in, W_c, b_c, w_v, W_out):
    from concourse.bass_utils import run_bass_kernel_spmd

    nc = get_nc()
    in_maps = _make_in_maps(
        dict(x=x, context=context, W_in=W_in, W_c=W_c, b_c=b_c, w_v=w_v, W_out=W_out)
    )
    res = run_bass_kernel_spmd(nc, in_maps, list(range(N_CORES)))
    h = np.concatenate([r["h_tilde"] for r in res.results], axis=0)
    aw = np.concatenate([r["attn_w"] for r in res.results], axis=0)
    return h, aw


if __name__ == "__main__":
    import reference as R

    inputs = {k: np.asarray(v) for k, v in R.setup_inputs().items()}
    h, aw = kernel(**inputs)
    print(h.shape, aw.shape, h.dtype, aw.dtype)
